# revision 3
# baseline (speedup 1.0000x reference)
"""LDS forward kernel for Trainium2 (8 NeuronCores, data-parallel over batch).

Math: the reference LDS with diagonal A and d_in == 1 is an exact causal
convolution plus a batch-independent bias:
    out[b,t,o] = sum_{d<=t} K[d,o] x[b,t-d] + bias[t,o]
    K[d,o]     = sum_s B[s] A[s]^d C[s,o]  (+ M[o,0,d-1] for d in 1..KX)
    bias[t,o]  = sum_s h0[s] A[s]^{t+1} C[s,o]
The stacked matrix G = [K; bias] (1024 x 512) is built from 512 decaying
exponentials, so it is numerically low rank: rank 32 reproduces it to
~3e-6.  Host computes (f64) G = U S V^T and splits factors U' (conv
kernels, 512 x 32), P' (bias coefficients, 512 x 32), W' (rank -> output
expansion, 32 x 512).

Device kernel per core (32 batch rows = 8 groups "bg" of 4 interleaved
rows, in 2 quads of 4 bgs):
  stage 1 (conv to rank space): per (quad, t-block j, bg-strip g) a PSUM
    chain over 32-lag chunks dc: psc[32g+rho, (j,tau,b)] += U32rev[dc]^T
    mega32, where mega32[32g+k, tau, b] = xpad[row, tau+k] holds 32
    host-materialized shifted copies of the signal.  The 16 chains of a
    quad occupy the 16 diagonal 32x32 tiles of the PE array
    (tile_position=(32g,32g), 4x4 row+col tiling -> concurrent chains);
    emission is round-robin over chains so the PE pipeline stays dense.
  evict: one VectorE tensor_add per quad adds the bias coefficients P
    and writes c (rank space, bf16) to SBUF.
  stage 2 (expand): out[o, (tau,b)] = W'[:,ob]^T c with contraction 32;
    the 4 bgs sit in the 4 32-row strips of the array (row tiling).
    PSUM tiles pair two bgs [128, 1024]; eviction (f32->bf16 cast)
    alternates VectorE / ScalarE (ScalarE slightly favoured).
  store: one DMA per (quad, ob, j-pair): [128 o-partitions, 8KB
    contiguous runs] to a PE-native DRAM layout; host transposes back.
"""

import numpy as np
import ml_dtypes

BSZ, T, D_IN = 256, 512, 1
S, O, KX = 512, 512, 5
NCORES = 8
BLOC = BSZ // NCORES        # 32 batch rows per core
NBG = BLOC // 4             # 8 groups of 4 batch rows
XPW = 640                   # padded signal width: 127 zeros + 512 + 1 slack
MW = 608                    # mega32 window width
R = 32                      # rank of the factored kernel

_prog_cache = {}
LAST_RESULTS = None         # BassKernelResults of the most recent run


def _build_program():
    import concourse.bacc as bacc
    import concourse.bass as bass
    import concourse.mybir as mybir
    from concourse.tile import TileContext

    f32 = mybir.dt.float32
    bf16 = mybir.dt.bfloat16

    nc = bacc.Bacc("TRN2", target_bir_lowering=False, debug=False)
    # mega32[quad, 32g+k, tau, b] = xpad[(quad*4+g)*4+b, tau + k]
    mega = nc.dram_tensor("mega", [2, 128, MW, 4], bf16, kind="ExternalInput")
    urev = nc.dram_tensor("urev", [128, 16, R], bf16, kind="ExternalInput")
    wrep = nc.dram_tensor("wrep", [128, 4, 128], bf16, kind="ExternalInput")
    psb = nc.dram_tensor("psb", [128, 4, 128, 4], bf16, kind="ExternalInput")
    # out[quad, ob, o, (j, gp, gi, tau, b)]
    out = nc.dram_tensor("out", [2, 4, 128, 8192], bf16, kind="ExternalOutput")

    with TileContext(nc) as tc:
        with (
            tc.tile_pool(name="consts", bufs=1) as cpool,
            tc.tile_pool(name="mega", bufs=2) as mpool,
            tc.tile_pool(name="csb", bufs=2) as cspool,
            tc.tile_pool(name="osb", bufs=3) as opool,
            tc.tile_pool(name="ps1", bufs=1, space="PSUM") as p1pool,
            tc.tile_pool(name="ps2", bufs=2, space="PSUM") as p2pool,
        ):
            # Consts + mega on the sync (SP HWDGE) ring ahead of the output
            # stores; psb rides the gpsimd (SWDGE) ring.
            urev_sb = cpool.tile([128, 16, R], bf16, tag="urev")
            nc.sync.dma_start(out=urev_sb[:], in_=urev.ap())
            wrep_sb = cpool.tile([128, 4, 128], bf16, tag="wrep")
            nc.sync.dma_start(out=wrep_sb[:], in_=wrep.ap())
            psb_sb = cpool.tile([128, 4, 128, 4], bf16, tag="psb")
            nc.gpsimd.dma_start(out=psb_sb[:], in_=psb.ap())
            psb_f = psb_sb[:].rearrange("p j t b -> p (j t b)")

            megas = []
            for quad in range(2):
                mg = mpool.tile([128, MW, 4], bf16, tag="mega")
                nc.sync.dma_start(out=mg[:], in_=mega.ap()[quad])
                megas.append(mg)

            evict_i = 0
            for quad in range(2):
                mf = megas[quad][:].rearrange("p t b -> p (t b)")
                c_sb = cspool.tile([128, 4 * T], bf16, tag="csb")
                psc = p1pool.tile([128, 2048], f32, tag="psc")
                # round-robin over the 16 (j, g) chains; chain j has 4j+4
                # lag-chunk steps.
                for dc in range(16):
                    for j in range(4):
                        if dc >= 4 * j + 4:
                            continue
                        base = (j * 128 - 32 * dc + 96) * 4
                        for g in range(4):
                            nc.tensor.matmul(
                                psc[32 * g : 32 * g + 32, j * 512 : j * 512 + 512],
                                urev_sb[32 * g : 32 * g + 32, dc, :],
                                mf[32 * g : 32 * g + 32, base : base + 512],
                                start=(dc == 0),
                                stop=(dc == 4 * j + 3),
                                tile_position=(32 * g, 32 * g),
                            )
                nc.vector.tensor_add(out=c_sb[:], in0=psc[:], in1=psb_f[:])
                for ob in range(4):
                    osb = opool.tile([128, 8192], bf16, tag="osb")
                    for jp in range(2):
                        for jh in range(2):
                            j = jp * 2 + jh
                            for gp in range(2):
                                pso = p2pool.tile([128, 1024], f32, tag="pso")
                                for gi in range(2):
                                    g = gp * 2 + gi
                                    nc.tensor.matmul(
                                        pso[:, gi * 512 : gi * 512 + 512],
                                        wrep_sb[32 * g : 32 * g + 32, ob, :],
                                        c_sb[32 * g : 32 * g + 32, j * 512 : j * 512 + 512],
                                        start=True,
                                        stop=True,
                                        tile_position=(32 * g, 0),
                                    )
                                dst = osb[:, j * 2048 + gp * 1024 : j * 2048 + gp * 1024 + 1024]
                                if evict_i % 16 < 7:
                                    nc.vector.tensor_copy(out=dst, in_=pso[:])
                                else:
                                    nc.scalar.copy(out=dst, in_=pso[:])
                                evict_i += 1
                        ddst = bass.AP(
                            out,
                            (quad * 4 + ob) * 128 * 8192 + jp * 4096,
                            [[8192, 128], [1, 4096]],
                        )
                        nc.sync.dma_start(out=ddst, in_=osb[:, jp * 4096 : jp * 4096 + 4096])
    nc.compile()
    return nc


def _get_program():
    if "p" not in _prog_cache:
        _prog_cache["p"] = _build_program()
    return _prog_cache["p"]


def host_prep(inputs, A, B, C, M, h0):
    """f64 host precompute: rank-R factors of [K; bias] + shifted windows."""
    x = inputs[:, :, 0].astype(np.float64)          # [BSZ, T]
    A64 = A.astype(np.float64)
    B64 = B.astype(np.float64)
    C64 = C.astype(np.float64)
    M64 = M.astype(np.float64)
    h64 = h0.astype(np.float64)

    Apow = A64[None, :] ** np.arange(T + 1)[:, None]      # [T+1, S]
    K = (B64[0][None, :] * Apow[:T]) @ C64                # [T, O]
    K[1 : KX + 1, :] += M64[:, 0, :].T                    # AR taps, lags 1..KX
    bias = (h64[None, :] * Apow[1 : T + 1]) @ C64         # [T, O]

    G = np.concatenate([K, bias], axis=0)                 # [2T, O]
    Ug, s, Vt = np.linalg.svd(G, full_matrices=False)
    sc = np.sqrt(s[:R])
    U = Ug[:T, :R] * sc                                   # [T, R] conv kernels
    P = Ug[T:, :R] * sc                                   # [T, R] bias coeffs
    W = Vt[:R] * sc[:, None]                              # [R, O]

    bf = ml_dtypes.bfloat16
    # urev[32g+k, dc, rho] = U[32dc + 31 - k, rho]  (replicated across g)
    u32 = np.ascontiguousarray(U.reshape(16, 32, R)[:, ::-1, :]).transpose(1, 0, 2)
    urev = np.ascontiguousarray(np.tile(u32, (4, 1, 1))).astype(bf)  # [128,16,R]
    wrep = np.ascontiguousarray(np.tile(W.reshape(R, 4, 128), (4, 1, 1))).astype(bf)
    # psb[p, j, tau, b] = P[j*128+tau, p % 32]
    psb = np.ascontiguousarray(
        np.tile(P.T.reshape(R, 4, 128)[:, :, :, None], (4, 1, 1, 4))
    ).astype(bf)                                          # [128, 4, 128, 4]

    xpad = np.zeros((BSZ, XPW), np.float32)
    xpad[:, 127 : 127 + T] = x
    xpad = xpad.astype(bf)                                # [BSZ, XPW]
    # mega32[core, quad, g*32+k, tau, b] = xpad[core*32 + (quad*4+g)*4 + b, tau+k]
    sw = np.lib.stride_tricks.sliding_window_view(xpad, MW, axis=1)  # [BSZ,33,MW]
    sw = sw[:, :32, :].reshape(NCORES, 2, 4, 4, 32, MW)   # [c, quad, g, b, k, tau]
    mega = np.ascontiguousarray(sw.transpose(0, 1, 2, 4, 5, 3))  # [c,quad,g,k,tau,b]
    mega = mega.reshape(NCORES, 2, 128, MW, 4)
    return mega, urev, wrep, psb


def kernel(inputs, A, B, C, M, h0):
    global LAST_RESULTS
    from concourse.bass_utils import run_bass_kernel_spmd

    mega, urev, wrep, psb = host_prep(inputs, A, B, C, M, h0)
    nc = _get_program()
    in_maps = [
        {"mega": mega[c], "urev": urev, "wrep": wrep, "psb": psb}
        for c in range(NCORES)
    ]
    res = run_bass_kernel_spmd(nc, in_maps, core_ids=list(range(NCORES)))
    LAST_RESULTS = res
    outs = []
    for r in res.results:
        arr = r["out"].reshape(2, 4, 128, 4, 2, 2, 128, 4)
        # [quad, ob, o, j, gp, gi, tau, b] -> [quad, gp, gi, b, j, tau, ob, o]
        arr = arr.transpose(0, 4, 5, 7, 3, 6, 1, 2).reshape(BLOC, T, O)
        outs.append(arr.astype(np.float32))
    return np.concatenate(outs, axis=0)


# revision 6
# speedup vs baseline: 1.0601x; 1.0601x over previous
"""LDS forward kernel for Trainium2 (8 NeuronCores, data-parallel over batch).

Math: the reference LDS with diagonal A and d_in == 1 is an exact causal
convolution plus a batch-independent bias:
    out[b,t,o] = sum_{d<=t} K[d,o] x[b,t-d] + bias[t,o]
    K[d,o]     = sum_s B[s] A[s]^d C[s,o]  (+ M[o,0,d-1] for d in 1..KX)
    bias[t,o]  = sum_s h0[s] A[s]^{t+1} C[s,o]
The stacked matrix G = [K; bias] (1024 x 512) is built from 512 decaying
exponentials, so it is numerically low rank: rank 32 reproduces it to
~3e-6.  Host computes (f64) G = U S V^T and splits factors U' (conv
kernels, 512 x 32), P' (bias coefficients, 512 x 32), W' (rank -> output
expansion, 32 x 512).

Device kernel per core (32 batch rows = 8 groups "bg" of 4 interleaved
rows, in 2 quads of 4 bgs):
  stage 1 (conv to rank space): per (quad, t-block j, bg-strip g) a PSUM
    chain over 32-lag chunks dc: psc[32g+rho, (j,tau,b)] += U32rev[dc]^T
    mega32, where mega32[32g+k, tau, b] = xpad[row, tau+k] holds 32
    host-materialized shifted copies of the signal.  The 16 chains of a
    quad occupy the 16 diagonal 32x32 tiles of the PE array
    (tile_position=(32g,32g), 4x4 row+col tiling -> concurrent chains);
    emission is round-robin over chains so the PE pipeline stays dense.
  evict: one VectorE tensor_add per quad adds the bias coefficients P
    and writes c (rank space, bf16) to SBUF.
  stage 2 (expand): out[o, (tau,b)] = W'[:,ob]^T c with contraction 32;
    the 4 bgs sit in the 4 32-row strips of the array (row tiling).
    PSUM tiles pair two bgs [128, 1024]; eviction (f32->bf16 cast)
    alternates VectorE / ScalarE (ScalarE slightly favoured).
  store: one DMA per (quad, ob, j-pair): [128 o-partitions, 8KB
    contiguous runs] to a PE-native DRAM layout; host transposes back.
"""

import numpy as np
import ml_dtypes

BSZ, T, D_IN = 256, 512, 1
S, O, KX = 512, 512, 5
NCORES = 8
BLOC = BSZ // NCORES        # 32 batch rows per core
NBG = BLOC // 4             # 8 groups of 4 batch rows
XPW = 640                   # padded signal width: 127 zeros + 512 + 1 slack
MW = 608                    # mega32 window width
R = 32                      # rank of the factored kernel

_prog_cache = {}
LAST_RESULTS = None         # BassKernelResults of the most recent run


def _build_program():
    import concourse.bacc as bacc
    import concourse.bass as bass
    import concourse.mybir as mybir
    from concourse.tile import TileContext

    f32 = mybir.dt.float32
    bf16 = mybir.dt.bfloat16

    nc = bacc.Bacc("TRN2", target_bir_lowering=False, debug=False)
    # mega32[quad, 32g+k, tau, b] = xpad[(quad*4+g)*4+b, tau + k]
    mega = nc.dram_tensor("mega", [2, 128, MW, 4], bf16, kind="ExternalInput")
    urev = nc.dram_tensor("urev", [128, 16, R], bf16, kind="ExternalInput")
    wrep = nc.dram_tensor("wrep", [128, 4, 128], bf16, kind="ExternalInput")
    psb = nc.dram_tensor("psb", [128, 4, 128, 4], bf16, kind="ExternalInput")
    # out[quad, ob, o, (j, gp, gi, tau, b)]
    out = nc.dram_tensor("out", [2, 4, 128, 8192], bf16, kind="ExternalOutput")

    with TileContext(nc) as tc:
        with (
            tc.tile_pool(name="consts", bufs=1) as cpool,
            tc.tile_pool(name="mega", bufs=2) as mpool,
            tc.tile_pool(name="csb", bufs=2) as cspool,
            tc.tile_pool(name="osb", bufs=3) as opool,
            tc.tile_pool(name="ps1", bufs=1, space="PSUM") as p1pool,
            tc.tile_pool(name="ps2", bufs=2, space="PSUM") as p2pool,
        ):
            # Consts + mega on the sync (SP HWDGE) ring ahead of the output
            # stores; psb rides the gpsimd (SWDGE) ring.
            urev_sb = cpool.tile([128, 16, R], bf16, tag="urev")
            nc.sync.dma_start(out=urev_sb[:], in_=urev.ap())
            wrep_sb = cpool.tile([128, 4, 128], bf16, tag="wrep")
            nc.sync.dma_start(out=wrep_sb[:], in_=wrep.ap())
            psb_sb = cpool.tile([128, 4, 128, 4], bf16, tag="psb")
            nc.gpsimd.dma_start(out=psb_sb[:], in_=psb.ap())
            psb_f = psb_sb[:].rearrange("p j t b -> p (j t b)")

            megas = []
            for quad in range(2):
                mg = mpool.tile([128, MW, 4], bf16, tag="mega")
                nc.sync.dma_start(out=mg[:], in_=mega.ap()[quad])
                megas.append(mg)

            evict_i = 0
            for quad in range(2):
                mf = megas[quad][:].rearrange("p t b -> p (t b)")
                c_sb = cspool.tile([128, 4 * T], bf16, tag="csb")
                psc = p1pool.tile([128, 2048], f32, tag="psc")
                # 16 (g, j) chains on 16 distinct PE tiles: row strip = 32g
                # (signal partitions), col strip = 32j (psc partitions);
                # chain j has 4j+4 lag-chunk steps, emitted round-robin so
                # the PE pipeline stays dense.
                # psc[32j+rho, (g, tau, b)]
                for dc in range(16):
                    for j in range(4):
                        if dc >= 4 * j + 4:
                            continue
                        base = (j * 128 - 32 * dc + 96) * 4
                        for g in range(4):
                            nc.tensor.matmul(
                                psc[32 * j : 32 * j + 32, g * 512 : g * 512 + 512],
                                urev_sb[32 * g : 32 * g + 32, dc, :],
                                mf[32 * g : 32 * g + 32, base : base + 512],
                                start=(dc == 0),
                                stop=(dc == 4 * j + 3),
                                tile_position=(32 * g, 32 * j),
                            )
                nc.vector.tensor_add(out=c_sb[:], in0=psc[:], in1=psb_f[:])
                for ob in range(4):
                    osb = opool.tile([128, 8192], bf16, tag="osb")
                    for gh in range(2):
                        for g in (2 * gh, 2 * gh + 1):
                            for jp in range(2):
                                pso = p2pool.tile([128, 1024], f32, tag="pso")
                                for jh in range(2):
                                    j = jp * 2 + jh
                                    nc.tensor.matmul(
                                        pso[:, jh * 512 : jh * 512 + 512],
                                        wrep_sb[32 * j : 32 * j + 32, ob, :],
                                        c_sb[32 * j : 32 * j + 32, g * 512 : g * 512 + 512],
                                        start=True,
                                        stop=True,
                                        tile_position=(32 * j, 0),
                                    )
                                dst = osb[:, g * 2048 + jp * 1024 : g * 2048 + jp * 1024 + 1024]
                                if evict_i % 16 < 7:
                                    nc.vector.tensor_copy(out=dst, in_=pso[:])
                                else:
                                    nc.scalar.copy(out=dst, in_=pso[:])
                                evict_i += 1
                        ddst = bass.AP(
                            out,
                            (quad * 4 + ob) * 128 * 8192 + gh * 4096,
                            [[8192, 128], [1, 4096]],
                        )
                        nc.sync.dma_start(out=ddst, in_=osb[:, gh * 4096 : gh * 4096 + 4096])
    nc.compile()
    return nc


def _get_program():
    if "p" not in _prog_cache:
        _prog_cache["p"] = _build_program()
    return _prog_cache["p"]


def host_prep(inputs, A, B, C, M, h0):
    """f64 host precompute: rank-R factors of [K; bias] + shifted windows."""
    x = inputs[:, :, 0].astype(np.float64)          # [BSZ, T]
    A64 = A.astype(np.float64)
    B64 = B.astype(np.float64)
    C64 = C.astype(np.float64)
    M64 = M.astype(np.float64)
    h64 = h0.astype(np.float64)

    Apow = A64[None, :] ** np.arange(T + 1)[:, None]      # [T+1, S]
    K = (B64[0][None, :] * Apow[:T]) @ C64                # [T, O]
    K[1 : KX + 1, :] += M64[:, 0, :].T                    # AR taps, lags 1..KX
    bias = (h64[None, :] * Apow[1 : T + 1]) @ C64         # [T, O]

    G = np.concatenate([K, bias], axis=0)                 # [2T, O]
    Ug, s, Vt = np.linalg.svd(G, full_matrices=False)
    sc = np.sqrt(s[:R])
    U = Ug[:T, :R] * sc                                   # [T, R] conv kernels
    P = Ug[T:, :R] * sc                                   # [T, R] bias coeffs
    W = Vt[:R] * sc[:, None]                              # [R, O]

    bf = ml_dtypes.bfloat16
    # urev[32g+k, dc, rho] = U[32dc + 31 - k, rho]  (replicated across g)
    u32 = np.ascontiguousarray(U.reshape(16, 32, R)[:, ::-1, :]).transpose(1, 0, 2)
    urev = np.ascontiguousarray(np.tile(u32, (4, 1, 1))).astype(bf)  # [128,16,R]
    wrep = np.ascontiguousarray(np.tile(W.reshape(R, 4, 128), (4, 1, 1))).astype(bf)
    # psb[32j+rho, g, tau, b] = P[j*128+tau, rho]  (g/b-independent)
    pjt = P.reshape(4, 128, R).transpose(0, 2, 1).reshape(128, 128)  # [(j,rho), tau]
    psb = np.ascontiguousarray(
        np.tile(pjt[:, None, :, None], (1, 4, 1, 4))
    ).astype(bf)                                          # [128, 4, 128, 4]

    xpad = np.zeros((BSZ, XPW), np.float32)
    xpad[:, 127 : 127 + T] = x
    xpad = xpad.astype(bf)                                # [BSZ, XPW]
    # mega32[core, quad, g*32+k, tau, b] = xpad[core*32 + (quad*4+g)*4 + b, tau+k]
    sw = np.lib.stride_tricks.sliding_window_view(xpad, MW, axis=1)  # [BSZ,33,MW]
    sw = sw[:, :32, :].reshape(NCORES, 2, 4, 4, 32, MW)   # [c, quad, g, b, k, tau]
    mega = np.ascontiguousarray(sw.transpose(0, 1, 2, 4, 5, 3))  # [c,quad,g,k,tau,b]
    mega = mega.reshape(NCORES, 2, 128, MW, 4)
    return mega, urev, wrep, psb


def kernel(inputs, A, B, C, M, h0):
    global LAST_RESULTS
    from concourse.bass_utils import run_bass_kernel_spmd

    mega, urev, wrep, psb = host_prep(inputs, A, B, C, M, h0)
    nc = _get_program()
    in_maps = [
        {"mega": mega[c], "urev": urev, "wrep": wrep, "psb": psb}
        for c in range(NCORES)
    ]
    res = run_bass_kernel_spmd(nc, in_maps, core_ids=list(range(NCORES)))
    LAST_RESULTS = res
    outs = []
    for r in res.results:
        arr = r["out"].reshape(2, 4, 128, 4, 2, 2, 128, 4)
        # [quad, ob, o, g, jp, jh, tau, b] -> [quad, g, b, jp, jh, tau, ob, o]
        arr = arr.transpose(0, 3, 7, 4, 5, 6, 1, 2).reshape(BLOC, T, O)
        outs.append(arr.astype(np.float32))
    return np.concatenate(outs, axis=0)
